# revision 5
# baseline (speedup 1.0000x reference)
"""Trainium2 Bass kernel for CombinedTemporalFocalBCELoss.

Math (exact rewrite of the reference):
  u = x*(2t-1); pt = sigmoid(u); bce = -ln(pt); q = 1-pt
  combined = 0.5*bce + 0.5*(-0.25)*q^2*ln(pt+eps)  ~= 0.125*(q^2+4)*bce
  weight = 1 - 0.2*m,  m = t AND any(t[i-5:i] == 1)
  out = mean(combined * weight)

Device computes, per core (chunk of N/8 elements, data-parallel with a
5-element targets halo):
  C_stored = (q^2 + 4) * ln(pt)            (= -8 * combined)
  acc1[p, tile] = sum_free C_stored        (fused STT accumulation)
  acc2          = sum m * C_stored         (PE ones-matmul reduction)
Host: mean = sum_cores -0.125*(sum acc1 - 0.2*sum acc2) / N
"""

import numpy as np

N_TOTAL = 16_777_216
N_CORES = 8
CHUNK = N_TOTAL // N_CORES      # 2_097_152
P = 128
F = 2048
NT = CHUNK // (P * F)           # 8
HALO = 5

_cache = {}


def _build_nc(reps=1):
    import concourse.bacc as bacc
    import concourse.mybir as mybir
    from concourse.tile import TileContext
    from concourse.ap import AP

    f32 = mybir.dt.float32
    bf16 = mybir.dt.bfloat16
    i32 = mybir.dt.int32
    AF = mybir.ActivationFunctionType
    Alu = mybir.AluOpType

    nc = bacc.Bacc("TRN2", target_bir_lowering=False, debug=False,
                   num_devices=N_CORES)

    x_in = nc.dram_tensor("x", [CHUNK], f32, kind="ExternalInput")
    ext_in = nc.dram_tensor("ext", [CHUNK + HALO], i32, kind="ExternalInput")
    o_acc1 = nc.dram_tensor("acc1", [P, NT], f32, kind="ExternalOutput").ap()
    o_acc2 = nc.dram_tensor("acc2", [1, 512], f32, kind="ExternalOutput").ap()

    x_view = x_in.ap().rearrange("(n p f) -> n p f", p=P, f=F)

    with TileContext(nc) as tc:
        with (
            tc.tile_pool(name="io", bufs=3) as io,
            tc.tile_pool(name="work", bufs=2) as work,
            tc.tile_pool(name="mpool", bufs=NT) as mpool,
            tc.tile_pool(name="rpool", bufs=NT) as rpool,
            tc.tile_pool(name="accp", bufs=1) as accp,
            tc.tile_pool(name="psum", bufs=1, space="PSUM") as psump,
        ):
            ones = accp.tile([P, 1], bf16, tag="ones")
            nc.vector.memset(ones[:], 1.0)
            acc1 = accp.tile([P, NT], f32, tag="acc1")
            psum = psump.tile([1, 512], f32)

            for rep in range(reps):
                m_tiles = []
                r_tiles = []
                # phase 1: mask path + h + sigmoid (sigmoid table set)
                for i in range(NT):
                    xb = io.tile([P, F], bf16, tag="x")
                    nc.gpsimd.dma_start(out=xb[:], in_=x_view[i])
                    e = io.tile([P, F + HALO], bf16, tag="e")
                    nc.gpsimd.dma_start(
                        out=e[:],
                        in_=AP(ext_in, i * P * F, [[F, P], [1, F + HALO]]))

                    A = work.tile([P, F + 3], bf16, tag="A")
                    nc.gpsimd.tensor_add(out=A[:], in0=e[:, 0:F + 3],
                                         in1=e[:, 1:F + 4])
                    B = work.tile([P, F + 1], bf16, tag="B")
                    nc.gpsimd.tensor_add(out=B[:], in0=A[:, 0:F + 1],
                                         in1=A[:, 2:F + 3])
                    w = work.tile([P, F], bf16, tag="w")
                    nc.vector.tensor_add(out=w[:], in0=B[:, 0:F],
                                         in1=e[:, 4:F + 4])

                    m = mpool.tile([P, F], bf16, tag="m")
                    nc.vector.scalar_tensor_tensor(
                        out=m[:], in0=w[:], scalar=1.0,
                        in1=e[:, HALO:F + HALO],
                        op0=Alu.min, op1=Alu.mult)
                    m_tiles.append(m)

                    h = work.tile([P, F], bf16, tag="h")
                    nc.vector.scalar_tensor_tensor(
                        out=h[:], in0=e[:, HALO:F + HALO], scalar=0.5,
                        in1=xb[:], op0=Alu.subtract, op1=Alu.mult)

                    r = rpool.tile([P, F], bf16, tag="r")
                    nc.scalar.activation(r[:], h[:], AF.Sigmoid, scale=2.0)
                    r_tiles.append(r)

                # phase 2: square+ln (natural_log set), C, m*C, PE reduce
                for i in range(NT):
                    q2 = work.tile([P, F], bf16, tag="q2")
                    nc.scalar.activation(q2[:], r_tiles[i][:], AF.Square,
                                         scale=-1.0, bias=1.0)
                    nb = work.tile([P, F], bf16, tag="nb")
                    nc.scalar.activation(nb[:], r_tiles[i][:], AF.Ln)

                    C = work.tile([P, F], bf16, tag="C")
                    nc.vector.scalar_tensor_tensor(
                        out=C[:], in0=q2[:], scalar=4.0, in1=nb[:],
                        op0=Alu.add, op1=Alu.mult,
                        accum_out=acc1[:, i:i + 1])

                    mC = work.tile([P, F], bf16, tag="mC")
                    nc.vector.tensor_mul(out=mC[:], in0=m_tiles[i][:],
                                         in1=C[:])

                    for j in range(F // 512):
                        nc.tensor.matmul(
                            out=psum[0:1, :],
                            lhsT=ones[:, 0:1],
                            rhs=mC[:, j * 512:(j + 1) * 512],
                            start=(i == 0 and j == 0),
                            stop=(i == NT - 1 and j == F // 512 - 1),
                        )

            acc2_sb = accp.tile([1, 512], f32, tag="acc2sb")
            nc.vector.tensor_copy(out=acc2_sb[:], in_=psum[0:1, :])
            nc.sync.dma_start(out=o_acc1, in_=acc1[:])
            nc.sync.dma_start(out=o_acc2, in_=acc2_sb[:])

    nc.compile()
    return nc


def _get_nc(reps=1):
    key = ("nc", reps)
    if key not in _cache:
        _cache[key] = _build_nc(reps)
    return _cache[key]


def _make_in_maps(outputs, targets):
    in_maps = []
    for c in range(N_CORES):
        lo, hi = c * CHUNK, (c + 1) * CHUNK
        halo = (np.zeros(HALO, np.int32) if c == 0
                else targets[lo - HALO:lo])
        ext = np.concatenate([halo, targets[lo:hi]]).astype(np.int32)
        in_maps.append({
            "x": np.ascontiguousarray(outputs[lo:hi], dtype=np.float32),
            "ext": ext,
        })
    return in_maps


def _combine(results):
    total = 0.0
    for res in results:
        a1 = np.asarray(res["acc1"], np.float64).sum()
        a2 = np.asarray(res["acc2"], np.float64).sum()
        total += -0.125 * (a1 - 0.2 * a2)
    return np.float32(total / N_TOTAL)


def kernel(outputs: np.ndarray, targets: np.ndarray) -> np.ndarray:
    from concourse.bass_utils import run_bass_kernel_spmd

    outputs = np.asarray(outputs)
    targets = np.asarray(targets)
    nc = _get_nc()
    res = run_bass_kernel_spmd(nc, _make_in_maps(outputs, targets),
                               core_ids=list(range(N_CORES)))
    return _combine(res.results)


def time_device(outputs, targets, reps=21, iters=3):
    """Estimate per-invocation device time via the wall-clock delta between
    a reps=K build and the reps=1 build (axon RPC overhead cancels)."""
    import time as _time
    from concourse.bass_utils import run_bass_kernel_spmd

    in_maps = _make_in_maps(np.asarray(outputs), np.asarray(targets))

    def best(nc):
        ts = []
        for _ in range(iters):
            t0 = _time.perf_counter()
            run_bass_kernel_spmd(nc, in_maps, core_ids=list(range(N_CORES)))
            ts.append(_time.perf_counter() - t0)
        return min(ts)

    nc1 = _get_nc(1)
    ncK = _get_nc(reps)
    t1 = best(nc1)
    tK = best(ncK)
    dt_ns = (tK - t1) / (reps - 1) * 1e9
    return dt_ns, t1, tK
